# revision 45
# baseline (speedup 1.0000x reference)
"""MetaPathAggregator kernel for Trainium2 (8 NeuronCores, data-parallel).

Math: the reference module is linear in the four gathered feature rows:

    out[t] = T_mi[i0] + T_g1[i1] + T_g2[i2] + T_dr[i3]

with per-slot transformed tables T_k = feat_src(k) @ M_k (128x128 M built
from the weight products).  Indices are < 1000 (spec fill_max), so only
1024 table rows are live.

The per-token work is 4 random row-fetches + 3 adds.  Two independent
fetch paths run concurrently:

  * X-path (DMA descriptors): tokens [0, XT) gather bf16 rows from DRAM
    T tables via dma_gather (1024 descriptors per instruction -- the
    ucode maximum), summed on DVE (bf16, 2x mode), stored bf16 (host
    upcasts exactly).
  * Y-path (Pool compute): tokens [XT, 16384) gather via ap_gather from
    SBUF transposed tables T^T [feat, row] (d=1: one f32 per channel per
    token), summed on DVE in [feat, tok] space, transposed back 128x128
    on PE into PSUM, copied to SBUF on ACT, stored f32.

The Pool engine runs both the X-path descriptor generation (~1.3ns/row)
and the Y-path ap_gather (~1.4ns/row); the DMA engines carry the X-path
transfers (~1.4ns/row) plus all loads/stores.  The split is chosen so
Pool and DMA finish together (~90us each in the TimelineSim cost model).
"""

import numpy as np

P = 128          # partitions
F = 128          # input feature dim
H = 128          # output hidden dim
HH = 64          # half hidden
R = 1024         # padded table rows (indices < 1000)
NT = R // P      # 8 row-tiles per table
N_CORES = 8
B_PAIRS = 1024
BAG = 128
TOK = B_PAIRS * BAG // N_CORES   # 16384 tokens per core

CHD = 1024                       # tokens per dma_gather (ucode max descs)
XT = 10240                       # DMA-path tokens
NCHD = XT // CHD                 # 10 chunks
CPBD = CHD // P                  # 8 tokens per partition per X chunk

YT = TOK - XT                    # 6144 Pool-path tokens
CHP = 1536
NCHP = YT // CHP                 # 4 chunks
NBLK = CHP // P                  # 12 transpose blocks per Y chunk

# X chunks emitted after each Y chunk (keeps Pool busy in arrival order)
_XPLAN = [list(range(3 * r, min(3 * r + 3, NCHD))) for r in range(NCHP - 1)]
_XPLAN.append(list(range(3 * (NCHP - 1), NCHD)))

_CACHE = {}


def _build_module():
    import concourse.bacc as bacc
    import concourse.mybir as mybir
    import concourse.tile as tile
    from concourse.masks import make_identity
    from concourse.tile_rust import add_dep_helper

    f32 = mybir.dt.float32
    bf16 = mybir.dt.bfloat16
    i16 = mybir.dt.int16
    COPY = mybir.ActivationFunctionType.Copy

    nc = bacc.Bacc("TRN2", dynamic_dma_scratch_size=65536)

    feat_in = {
        "mi": nc.dram_tensor("feat_mi", [R, F], f32, kind="ExternalInput"),
        "ge": nc.dram_tensor("feat_ge", [R, F], f32, kind="ExternalInput"),
        "dr": nc.dram_tensor("feat_dr", [R, F], f32, kind="ExternalInput"),
    }
    w_dd = nc.dram_tensor("w_dd", [H, F], f32, kind="ExternalInput")
    w_dg = nc.dram_tensor("w_dg", [H, F], f32, kind="ExternalInput")
    w_drug = nc.dram_tensor("w_drug", [HH, F], f32, kind="ExternalInput")
    w_dis = nc.dram_tensor("w_dis", [HH, F], f32, kind="ExternalInput")
    idx_d_in = nc.dram_tensor("idx_d", [P, 4, NCHD, CHD // 16], i16, kind="ExternalInput")
    idx_p_in = nc.dram_tensor("idx_p", [P, 4, NCHP, CHP // 16], i16, kind="ExternalInput")
    out_d = nc.dram_tensor("out_d", [XT, H], bf16, kind="ExternalOutput")
    out_p = nc.dram_tensor("out_p", [YT, H], f32, kind="ExternalOutput")

    with tile.TileContext(nc) as tc:
        with (
            tc.tile_pool(name="const", bufs=1) as cpool,
            tc.tile_pool(name="tdram", bufs=1, space="DRAM") as dpool,
            tc.tile_pool(name="gather", bufs=3) as gpool,
            tc.tile_pool(name="ypath", bufs=2) as ypool,
        ):
            ident = cpool.tile([P, P], f32)
            make_identity(nc, ident[:])

            idx_d = cpool.tile([P, 4, NCHD, CHD // 16], i16, tag="idxd")
            nc.sync.dma_start(idx_d[:], idx_d_in[:, :, :, :])
            idx_p = cpool.tile([P, 4, NCHP, CHP // 16], i16, tag="idxp")
            nc.sync.dma_start(idx_p[:], idx_p_in[:, :, :, :])

            # DRAM scratch for the X-path bf16 tables
            t_dram = [dpool.tile([R, F], bf16, tag=f"t{k}", name=f"t_dram{k}")
                      for k in range(4)]

            # ---- load weights
            wdd_t = cpool.tile([H, F], f32, tag="wdd")
            nc.sync.dma_start(wdd_t[:], w_dd[:, :])
            wdg_t = cpool.tile([H, F], f32, tag="wdg")
            nc.sync.dma_start(wdg_t[:], w_dg[:, :])
            wdrug_t = cpool.tile([HH, F], f32, tag="wdrug")
            nc.sync.dma_start(wdrug_t[:], w_drug[:, :])
            wdis_t = cpool.tile([HH, F], f32, tag="wdis")
            nc.sync.dma_start(wdis_t[:], w_dis[:, :])

            t_store = [None] * 4
            ttpose = [cpool.tile([P, NT, P], f32, tag=f"tt{k}", name=f"ttpose{k}")
                      for k in range(4)]

            with (
                tc.tile_pool(name="prep", bufs=2) as ppool,
                tc.tile_pool(name="psum", bufs=4, space="PSUM") as pspool,
                tc.tile_pool(name="wps", bufs=2, space="PSUM") as wpool,
            ):
                # ---- C = Wdrug^T, D = Wdis^T  (PE transpose via identity)
                c_ps = wpool.tile([F, HH], f32, tag="tps")
                nc.tensor.transpose(out=c_ps[:], in_=wdrug_t[:], identity=ident[:HH, :HH])
                c_s = cpool.tile([F, HH], f32, tag="c_s")
                nc.vector.tensor_copy(out=c_s[:], in_=c_ps[:])

                d_ps = wpool.tile([F, HH], f32, tag="tps")
                nc.tensor.transpose(out=d_ps[:], in_=wdis_t[:], identity=ident[:HH, :HH])
                d_s = cpool.tile([F, HH], f32, tag="d_s")
                nc.vector.tensor_copy(out=d_s[:], in_=d_ps[:])

                # ---- A = Wdd^T @ Wdis^T, B = Wdg^T @ Wdrug^T
                a_ps = wpool.tile([F, HH], f32, tag="abps")
                nc.tensor.matmul(out=a_ps[:], lhsT=wdd_t[:], rhs=d_s[:], start=True, stop=True)
                b_ps = wpool.tile([F, HH], f32, tag="abps")
                nc.tensor.matmul(out=b_ps[:], lhsT=wdg_t[:], rhs=c_s[:], start=True, stop=True)

                # ---- assemble M matrices [F, H] and bf16 copies
                m = {k: cpool.tile([F, H], f32, tag=f"m_{k}", name=f"m_{k}")
                     for k in range(4)}
                nc.vector.tensor_scalar_mul(m[0][:, :HH], c_s[:], 0.5)
                nc.vector.tensor_scalar_mul(m[0][:, HH:], a_ps[:], 0.125)
                nc.vector.tensor_scalar_mul(m[1][:, :HH], c_s[:], 0.25)
                nc.vector.tensor_scalar_mul(m[1][:, HH:], a_ps[:], 0.125)
                nc.vector.tensor_scalar_mul(m[2][:, :HH], b_ps[:], 0.125)
                nc.vector.tensor_scalar_mul(m[2][:, HH:], d_s[:], 0.25)
                nc.vector.tensor_scalar_mul(m[3][:, :HH], b_ps[:], 0.125)
                nc.vector.tensor_scalar_mul(m[3][:, HH:], d_s[:], 0.5)
                m_bf = {k: cpool.tile([F, H], bf16, tag=f"mb_{k}", name=f"mb_{k}")
                        for k in range(4)}
                for k in range(4):
                    nc.vector.tensor_copy(out=m_bf[k][:], in_=m[k][:])

                # ---- per feature table: transpose row-tiles to bf16, then
                # T^T_k (f32, SBUF) for the Y-path and T_k (bf16, DRAM) for
                # the X-path.
                feat_slots = {"mi": [0], "ge": [1, 2], "dr": [3]}
                for name in ("mi", "ge", "dr"):
                    ft = ppool.tile([P, NT, F], f32, tag="feat", name=f"feat_{name}")
                    nc.sync.dma_start(
                        ft[:], feat_in[name][:, :].rearrange("(r p) f -> p r f", p=P)
                    )
                    fts = ppool.tile([P, NT, F], bf16, tag="ftT", name=f"ftT_{name}")
                    for g in range(2):          # groups of 4 row-tiles
                        tp = pspool.tile([P, 4, P], f32, tag="ps512")
                        for r4 in range(4):
                            nc.tensor.transpose(
                                out=tp[:, r4, :], in_=ft[:, g * 4 + r4, :],
                                identity=ident[:],
                            )
                        nc.vector.tensor_copy(out=fts[:, g * 4:(g + 1) * 4, :], in_=tp[:])

                    for k in feat_slots[name]:
                        # T^T blocks: out[h, p] = T[r*128+p, h]
                        for g in range(2):
                            mmt = pspool.tile([P, 4, P], f32, tag="ps512")
                            for r4 in range(4):
                                nc.tensor.matmul(
                                    out=mmt[:, r4, :], lhsT=m_bf[k][:],
                                    rhs=fts[:, g * 4 + r4, :],
                                    start=True, stop=True,
                                )
                            nc.scalar.activation(
                                out=ttpose[k][:, g * 4:(g + 1) * 4, :], in_=mmt[:],
                                func=COPY,
                            )
                        # T blocks: out[p, h] = T[r*128+p, h] -> bf16 staged
                        tstage = ppool.tile([P, NT, H], bf16, tag=f"tstage{k}",
                                            name=f"tstage{k}", bufs=1)
                        for g in range(2):
                            mm = pspool.tile([P, 4, P], f32, tag="ps512")
                            for r4 in range(4):
                                nc.tensor.matmul(
                                    out=mm[:, r4, :], lhsT=fts[:, g * 4 + r4, :],
                                    rhs=m_bf[k][:],
                                    start=True, stop=True,
                                )
                            nc.vector.tensor_copy(out=tstage[:, g * 4:(g + 1) * 4, :], in_=mm[:])
                        t_store[k] = nc.sync.dma_start(
                            t_dram[k][:, :].rearrange("(r p) f -> p r f", p=P),
                            tstage[:],
                        )

            # ---- main loop ------------------------------------------------
            xtiles = {}

            def emit_x_gathers(c):
                g = []
                for k in range(4):
                    gt = gpool.tile([P, CPBD, F], bf16, tag=f"g{k}", name=f"g{k}_{c}")
                    gi = nc.gpsimd.dma_gather(
                        gt[:], t_dram[k][:, :], idx_d[:, k, c, :], CHD, CHD, F,
                    )
                    add_dep_helper(gi.ins, t_store[k].ins,
                                   reason="gather after T store")
                    g.append(gt)
                xtiles[c] = g

            def emit_x_tail(c):
                g = xtiles.pop(c)
                nc.vector.tensor_add(g[0][:], g[0][:], g[1][:])
                nc.vector.tensor_add(g[2][:], g[2][:], g[3][:])
                nc.vector.tensor_add(g[0][:], g[0][:], g[2][:])
                nc.sync.dma_start(
                    out_d[c * CHD:(c + 1) * CHD, :].rearrange(
                        "(p s) h -> p s h", p=P),
                    g[0][:],
                )

            def emit_x_chunk(c):
                emit_x_gathers(c)
                emit_x_tail(c)

            # PSUM block-groups per Y chunk (NBLK blocks in groups of <=4)
            ygroups = []
            b0 = 0
            while b0 < NBLK:
                gw = min(4, NBLK - b0)
                ygroups.append((b0, gw))
                b0 += gw

            with (
                tc.tile_pool(name="ypsum", bufs=4, space="PSUM") as ypsum,
            ):
                for c in range(NCHP):
                    yt = []
                    for k in range(4):
                        t = ypool.tile([P, CHP], f32, tag=f"y{k}", name=f"y{k}_{c}")
                        nc.gpsimd.ap_gather(
                            t[:], ttpose[k][:], idx_p[:, k, c, :],
                            channels=P, num_elems=R, d=1, num_idxs=CHP,
                        )
                        yt.append(t)

                    # sum in [feat, tok] space (f32, in place into yt[0])
                    nc.vector.tensor_add(yt[0][:], yt[0][:], yt[1][:])
                    nc.vector.tensor_add(yt[2][:], yt[2][:], yt[3][:])
                    nc.vector.tensor_add(yt[0][:], yt[0][:], yt[2][:])

                    # transpose 128x128 blocks back to [tok, feat] via PE
                    yst = ypool.tile([P, NBLK, F], f32, tag="yst", name=f"yst_{c}")
                    for (gb, gw) in ygroups:
                        ps = ypsum.tile([P, gw, P], f32, tag=f"yps{gw}")
                        for b4 in range(gw):
                            b = gb + b4
                            nc.tensor.transpose(
                                out=ps[:, b4, :], in_=yt[0][:, b * P:(b + 1) * P],
                                identity=ident[:],
                            )
                        nc.scalar.activation(
                            out=yst[:, gb:gb + gw, :], in_=ps[:], func=COPY,
                        )
                    nc.sync.dma_start(
                        out_p[c * CHP:(c + 1) * CHP, :].rearrange(
                            "(b p) h -> p b h", p=P),
                        yst[:],
                    )

                    for xc in _XPLAN[c]:
                        emit_x_chunk(xc)

    nc.compile()
    return nc


def _wrap_idx(lin):
    """int16 index vector -> [128, n//16] wrapped/replicated gather layout."""
    wrapped = lin.reshape(-1, 16).T          # [16, n/16]
    return np.tile(wrapped, (8, 1))          # [128, n/16]


def _prep_inputs(feat_miRNA, feat_gene, feat_drug, W_drug_disease, W_disease_drug,
                 W_drug, W_dis, mp_ins):
    """Marshal full inputs into per-core in_maps (no arithmetic on values)."""
    def pad_rows(a):
        a = np.ascontiguousarray(np.asarray(a, dtype=np.float32))
        if a.shape[0] >= R:
            return np.ascontiguousarray(a[:R])
        out = np.zeros((R, a.shape[1]), dtype=np.float32)
        out[: a.shape[0]] = a
        return out

    f_mi = pad_rows(feat_miRNA)
    f_ge = pad_rows(feat_gene)
    f_dr = pad_rows(feat_drug)
    wdd = np.ascontiguousarray(np.asarray(W_drug_disease, np.float32))
    wdg = np.ascontiguousarray(np.asarray(W_disease_drug, np.float32))
    wdrug = np.ascontiguousarray(np.asarray(W_drug, np.float32))
    wdis = np.ascontiguousarray(np.asarray(W_dis, np.float32))

    mp = np.asarray(mp_ins)
    assert mp.shape == (B_PAIRS, BAG, 4), mp.shape

    # X-path: within chunk c, gather slot j holds token (j%128)*CPBD + j//128
    jd = np.arange(CHD)
    tok_of_jd = (jd % P) * CPBD + (jd // P)

    in_maps = []
    for core in range(N_CORES):
        mp_core = mp[core * (B_PAIRS // N_CORES): (core + 1) * (B_PAIRS // N_CORES)]
        mp_core = mp_core.reshape(TOK, 4).astype(np.int16)

        idx_d = np.empty((P, 4, NCHD, CHD // 16), dtype=np.int16)
        for c in range(NCHD):
            t = c * CHD + tok_of_jd
            for k in range(4):
                idx_d[:, k, c, :] = _wrap_idx(mp_core[t, k])

        idx_p = np.empty((P, 4, NCHP, CHP // 16), dtype=np.int16)
        for c in range(NCHP):
            t = XT + c * CHP + np.arange(CHP)       # sequential tokens
            for k in range(4):
                idx_p[:, k, c, :] = _wrap_idx(mp_core[t, k])

        in_maps.append(
            {
                "feat_mi": f_mi,
                "feat_ge": f_ge,
                "feat_dr": f_dr,
                "w_dd": wdd,
                "w_dg": wdg,
                "w_drug": wdrug,
                "w_dis": wdis,
                "idx_d": idx_d,
                "idx_p": idx_p,
            }
        )
    return in_maps


def _numpy_fallback(feat_miRNA, feat_gene, feat_drug, W_drug_disease,
                    W_disease_drug, W_drug, W_dis, mp_ins):
    mi = np.asarray(feat_miRNA, np.float32)[mp_ins[:, :, 0]]
    g1 = np.asarray(feat_gene, np.float32)[mp_ins[:, :, 1]]
    g2 = np.asarray(feat_gene, np.float32)[mp_ins[:, :, 2]]
    dr = np.asarray(feat_drug, np.float32)[mp_ins[:, :, 3]]
    wdd = np.asarray(W_drug_disease, np.float32)
    wdg = np.asarray(W_disease_drug, np.float32)
    wdrug = np.asarray(W_drug, np.float32)
    wdis = np.asarray(W_dis, np.float32)
    dis = ((((mi + g1) * 0.5) @ wdd.T + g2) * 0.5 + dr) * 0.5
    drug = ((((dr + g2) * 0.5) @ wdg.T + g1) * 0.5 + mi) * 0.5
    return np.concatenate([drug @ wdrug.T, dis @ wdis.T], axis=2)


def kernel(**inputs):
    mp = np.asarray(inputs["mp_ins"])
    if mp.max() >= R or mp.min() < 0:
        # outside the spec's index range; fall back to exact host compute
        return _numpy_fallback(**inputs)

    from concourse.bass_utils import run_bass_kernel_spmd

    if "nc" not in _CACHE:
        _CACHE["nc"] = _build_module()
    nc = _CACHE["nc"]

    in_maps = _prep_inputs(**inputs)
    res = run_bass_kernel_spmd(nc, in_maps, core_ids=list(range(N_CORES)))
    outs = []
    for r in res.results:
        full = np.empty((TOK, H), dtype=np.float32)
        full[:XT] = np.asarray(r["out_d"]).astype(np.float32)  # exact upcast
        full[XT:] = r["out_p"]
        outs.append(full)
    return np.concatenate(outs, axis=0).reshape(B_PAIRS, BAG, H)


if __name__ == "__main__":
    import reference

    inputs = {k: np.asarray(v) for k, v in reference.setup_inputs().items()}
    expected = np.asarray(reference.reference(**inputs))
    actual = kernel(**inputs)
    err = np.abs(actual - expected).max() / (np.abs(expected).max() + 1e-9)
    print("max abs err (scaled):", err)
    rel = np.linalg.norm(actual - expected) / np.linalg.norm(expected)
    print("Relative error:", rel)


# revision 47
# speedup vs baseline: 1.0258x; 1.0258x over previous
"""MetaPathAggregator kernel for Trainium2 (8 NeuronCores, data-parallel).

Math: the reference module is linear in the four gathered feature rows:

    out[t] = T_mi[i0] + T_g1[i1] + T_g2[i2] + T_dr[i3]

with per-slot transformed tables T_k = feat_src(k) @ M_k (128x128 M built
from the weight products).  Indices are < 1000 (spec fill_max), so only
1024 table rows are live.

The per-token work is 4 random row-fetches + 3 adds.  Two independent
fetch paths run concurrently:

  * X-path (DMA descriptors): tokens [0, XT) gather bf16 rows from DRAM
    T tables via dma_gather (1024 descriptors per instruction -- the
    ucode maximum), summed on DVE (bf16, 2x mode), stored bf16 (host
    upcasts exactly).
  * Y-path (Pool compute): tokens [XT, 16384) gather via ap_gather from
    SBUF transposed tables T^T [feat, row] (d=1: one f32 per channel per
    token), summed on DVE in [feat, tok] space, transposed back 128x128
    on PE into PSUM, copied to SBUF on ACT, stored f32.

The Pool engine runs both the X-path descriptor generation (~1.3ns/row)
and the Y-path ap_gather (~1.4ns/row); the DMA engines carry the X-path
transfers (~1.4ns/row) plus all loads/stores.  The split is chosen so
Pool and DMA finish together (~90us each in the TimelineSim cost model).
"""

import numpy as np

P = 128          # partitions
F = 128          # input feature dim
H = 128          # output hidden dim
HH = 64          # half hidden
R = 1024         # padded table rows (indices < 1000)
NT = R // P      # 8 row-tiles per table
N_CORES = 8
B_PAIRS = 1024
BAG = 128
TOK = B_PAIRS * BAG // N_CORES   # 16384 tokens per core

CHD = 1024                       # tokens per dma_gather (ucode max descs)
XT = 11264                       # DMA-path tokens
NCHD = XT // CHD                 # 11 chunks
CPBD = CHD // P                  # 8 tokens per partition per X chunk

YT = TOK - XT                    # 5120 Pool-path tokens
CHP = 1280
NCHP = YT // CHP                 # 4 chunks
NBLK = CHP // P                  # 10 transpose blocks per Y chunk

# X chunks emitted after each Y chunk (keeps Pool busy in arrival order)
_XPLAN = [list(range(3 * r, min(3 * r + 3, NCHD))) for r in range(NCHP - 1)]
_XPLAN.append(list(range(3 * (NCHP - 1), NCHD)))

_CACHE = {}


def _build_module():
    import concourse.bacc as bacc
    import concourse.mybir as mybir
    import concourse.tile as tile
    from concourse.masks import make_identity
    from concourse.tile_rust import add_dep_helper

    f32 = mybir.dt.float32
    bf16 = mybir.dt.bfloat16
    i16 = mybir.dt.int16
    COPY = mybir.ActivationFunctionType.Copy

    nc = bacc.Bacc("TRN2", dynamic_dma_scratch_size=65536)

    feat_in = {
        "mi": nc.dram_tensor("feat_mi", [R, F], f32, kind="ExternalInput"),
        "ge": nc.dram_tensor("feat_ge", [R, F], f32, kind="ExternalInput"),
        "dr": nc.dram_tensor("feat_dr", [R, F], f32, kind="ExternalInput"),
    }
    w_dd = nc.dram_tensor("w_dd", [H, F], f32, kind="ExternalInput")
    w_dg = nc.dram_tensor("w_dg", [H, F], f32, kind="ExternalInput")
    w_drug = nc.dram_tensor("w_drug", [HH, F], f32, kind="ExternalInput")
    w_dis = nc.dram_tensor("w_dis", [HH, F], f32, kind="ExternalInput")
    idx_d_in = nc.dram_tensor("idx_d", [P, 4, NCHD, CHD // 16], i16, kind="ExternalInput")
    idx_p_in = nc.dram_tensor("idx_p", [P, 4, NCHP, CHP // 16], i16, kind="ExternalInput")
    out_d = nc.dram_tensor("out_d", [XT, H], bf16, kind="ExternalOutput")
    out_p = nc.dram_tensor("out_p", [YT, H], f32, kind="ExternalOutput")

    with tile.TileContext(nc) as tc:
        with (
            tc.tile_pool(name="const", bufs=1) as cpool,
            tc.tile_pool(name="tdram", bufs=1, space="DRAM") as dpool,
            tc.tile_pool(name="gather", bufs=4) as gpool,
            tc.tile_pool(name="ypath", bufs=2) as ypool,
        ):
            ident = cpool.tile([P, P], f32)
            make_identity(nc, ident[:])

            idx_d = cpool.tile([P, 4, NCHD, CHD // 16], i16, tag="idxd")
            nc.sync.dma_start(idx_d[:], idx_d_in[:, :, :, :])
            idx_p = cpool.tile([P, 4, NCHP, CHP // 16], i16, tag="idxp")
            nc.sync.dma_start(idx_p[:], idx_p_in[:, :, :, :])

            # DRAM scratch for the X-path bf16 tables
            t_dram = [dpool.tile([R, F], bf16, tag=f"t{k}", name=f"t_dram{k}")
                      for k in range(4)]

            # ---- load weights
            wdd_t = cpool.tile([H, F], f32, tag="wdd")
            nc.sync.dma_start(wdd_t[:], w_dd[:, :])
            wdg_t = cpool.tile([H, F], f32, tag="wdg")
            nc.sync.dma_start(wdg_t[:], w_dg[:, :])
            wdrug_t = cpool.tile([HH, F], f32, tag="wdrug")
            nc.sync.dma_start(wdrug_t[:], w_drug[:, :])
            wdis_t = cpool.tile([HH, F], f32, tag="wdis")
            nc.sync.dma_start(wdis_t[:], w_dis[:, :])

            t_store = [None] * 4
            ttpose = [cpool.tile([P, NT, P], f32, tag=f"tt{k}", name=f"ttpose{k}")
                      for k in range(4)]

            with (
                tc.tile_pool(name="prep", bufs=2) as ppool,
                tc.tile_pool(name="psum", bufs=4, space="PSUM") as pspool,
                tc.tile_pool(name="wps", bufs=2, space="PSUM") as wpool,
            ):
                # ---- C = Wdrug^T, D = Wdis^T  (PE transpose via identity)
                c_ps = wpool.tile([F, HH], f32, tag="tps")
                nc.tensor.transpose(out=c_ps[:], in_=wdrug_t[:], identity=ident[:HH, :HH])
                c_s = cpool.tile([F, HH], f32, tag="c_s")
                nc.vector.tensor_copy(out=c_s[:], in_=c_ps[:])

                d_ps = wpool.tile([F, HH], f32, tag="tps")
                nc.tensor.transpose(out=d_ps[:], in_=wdis_t[:], identity=ident[:HH, :HH])
                d_s = cpool.tile([F, HH], f32, tag="d_s")
                nc.vector.tensor_copy(out=d_s[:], in_=d_ps[:])

                # ---- A = Wdd^T @ Wdis^T, B = Wdg^T @ Wdrug^T
                a_ps = wpool.tile([F, HH], f32, tag="abps")
                nc.tensor.matmul(out=a_ps[:], lhsT=wdd_t[:], rhs=d_s[:], start=True, stop=True)
                b_ps = wpool.tile([F, HH], f32, tag="abps")
                nc.tensor.matmul(out=b_ps[:], lhsT=wdg_t[:], rhs=c_s[:], start=True, stop=True)

                # ---- assemble M matrices [F, H] and bf16 copies
                m = {k: cpool.tile([F, H], f32, tag=f"m_{k}", name=f"m_{k}")
                     for k in range(4)}
                nc.vector.tensor_scalar_mul(m[0][:, :HH], c_s[:], 0.5)
                nc.vector.tensor_scalar_mul(m[0][:, HH:], a_ps[:], 0.125)
                nc.vector.tensor_scalar_mul(m[1][:, :HH], c_s[:], 0.25)
                nc.vector.tensor_scalar_mul(m[1][:, HH:], a_ps[:], 0.125)
                nc.vector.tensor_scalar_mul(m[2][:, :HH], b_ps[:], 0.125)
                nc.vector.tensor_scalar_mul(m[2][:, HH:], d_s[:], 0.25)
                nc.vector.tensor_scalar_mul(m[3][:, :HH], b_ps[:], 0.125)
                nc.vector.tensor_scalar_mul(m[3][:, HH:], d_s[:], 0.5)
                m_bf = {k: cpool.tile([F, H], bf16, tag=f"mb_{k}", name=f"mb_{k}")
                        for k in range(4)}
                for k in range(4):
                    nc.vector.tensor_copy(out=m_bf[k][:], in_=m[k][:])

                # ---- per feature table: transpose row-tiles to bf16, then
                # T^T_k (f32, SBUF) for the Y-path and T_k (bf16, DRAM) for
                # the X-path.
                feat_slots = {"mi": [0], "ge": [1, 2], "dr": [3]}
                for name in ("mi", "ge", "dr"):
                    ft = ppool.tile([P, NT, F], f32, tag="feat", name=f"feat_{name}")
                    nc.sync.dma_start(
                        ft[:], feat_in[name][:, :].rearrange("(r p) f -> p r f", p=P)
                    )
                    fts = ppool.tile([P, NT, F], bf16, tag="ftT", name=f"ftT_{name}")
                    for g in range(2):          # groups of 4 row-tiles
                        tp = pspool.tile([P, 4, P], f32, tag="ps512")
                        for r4 in range(4):
                            nc.tensor.transpose(
                                out=tp[:, r4, :], in_=ft[:, g * 4 + r4, :],
                                identity=ident[:],
                            )
                        nc.vector.tensor_copy(out=fts[:, g * 4:(g + 1) * 4, :], in_=tp[:])

                    for k in feat_slots[name]:
                        # T^T blocks: out[h, p] = T[r*128+p, h]
                        for g in range(2):
                            mmt = pspool.tile([P, 4, P], f32, tag="ps512")
                            for r4 in range(4):
                                nc.tensor.matmul(
                                    out=mmt[:, r4, :], lhsT=m_bf[k][:],
                                    rhs=fts[:, g * 4 + r4, :],
                                    start=True, stop=True,
                                )
                            nc.scalar.activation(
                                out=ttpose[k][:, g * 4:(g + 1) * 4, :], in_=mmt[:],
                                func=COPY,
                            )
                        # T blocks: out[p, h] = T[r*128+p, h] -> bf16 staged
                        tstage = ppool.tile([P, NT, H], bf16, tag=f"tstage{k}",
                                            name=f"tstage{k}", bufs=1)
                        for g in range(2):
                            mm = pspool.tile([P, 4, P], f32, tag="ps512")
                            for r4 in range(4):
                                nc.tensor.matmul(
                                    out=mm[:, r4, :], lhsT=fts[:, g * 4 + r4, :],
                                    rhs=m_bf[k][:],
                                    start=True, stop=True,
                                )
                            nc.vector.tensor_copy(out=tstage[:, g * 4:(g + 1) * 4, :], in_=mm[:])
                        t_store[k] = nc.sync.dma_start(
                            t_dram[k][:, :].rearrange("(r p) f -> p r f", p=P),
                            tstage[:],
                        )

            # ---- main loop ------------------------------------------------
            xtiles = {}

            def emit_x_gathers(c):
                g = []
                for k in range(4):
                    gt = gpool.tile([P, CPBD, F], bf16, tag=f"g{k}", name=f"g{k}_{c}")
                    gi = nc.gpsimd.dma_gather(
                        gt[:], t_dram[k][:, :], idx_d[:, k, c, :], CHD, CHD, F,
                    )
                    add_dep_helper(gi.ins, t_store[k].ins,
                                   reason="gather after T store")
                    g.append(gt)
                xtiles[c] = g

            def emit_x_tail(c):
                g = xtiles.pop(c)
                nc.vector.tensor_add(g[0][:], g[0][:], g[1][:])
                nc.vector.tensor_add(g[2][:], g[2][:], g[3][:])
                nc.vector.tensor_add(g[0][:], g[0][:], g[2][:])
                nc.sync.dma_start(
                    out_d[c * CHD:(c + 1) * CHD, :].rearrange(
                        "(p s) h -> p s h", p=P),
                    g[0][:],
                )

            def emit_x_chunk(c):
                emit_x_gathers(c)
                emit_x_tail(c)

            # PSUM block-groups per Y chunk (NBLK blocks in groups of <=4)
            ygroups = []
            b0 = 0
            while b0 < NBLK:
                gw = min(4, NBLK - b0)
                ygroups.append((b0, gw))
                b0 += gw

            with (
                tc.tile_pool(name="ypsum", bufs=4, space="PSUM") as ypsum,
            ):
                for c in range(NCHP):
                    yt = []
                    for k in range(4):
                        t = ypool.tile([P, CHP], f32, tag=f"y{k}", name=f"y{k}_{c}")
                        nc.gpsimd.ap_gather(
                            t[:], ttpose[k][:], idx_p[:, k, c, :],
                            channels=P, num_elems=R, d=1, num_idxs=CHP,
                        )
                        yt.append(t)

                    # sum in [feat, tok] space (f32, in place into yt[0])
                    nc.vector.tensor_add(yt[0][:], yt[0][:], yt[1][:])
                    nc.vector.tensor_add(yt[2][:], yt[2][:], yt[3][:])
                    nc.vector.tensor_add(yt[0][:], yt[0][:], yt[2][:])

                    # transpose 128x128 blocks back to [tok, feat] via PE
                    yst = ypool.tile([P, NBLK, F], f32, tag="yst", name=f"yst_{c}")
                    for (gb, gw) in ygroups:
                        ps = ypsum.tile([P, gw, P], f32, tag=f"yps{gw}")
                        for b4 in range(gw):
                            b = gb + b4
                            nc.tensor.transpose(
                                out=ps[:, b4, :], in_=yt[0][:, b * P:(b + 1) * P],
                                identity=ident[:],
                            )
                        nc.scalar.activation(
                            out=yst[:, gb:gb + gw, :], in_=ps[:], func=COPY,
                        )
                    nc.sync.dma_start(
                        out_p[c * CHP:(c + 1) * CHP, :].rearrange(
                            "(b p) h -> p b h", p=P),
                        yst[:],
                    )

                    for xc in _XPLAN[c]:
                        emit_x_chunk(xc)

    nc.compile()
    return nc


def _wrap_idx(lin):
    """int16 index vector -> [128, n//16] wrapped/replicated gather layout."""
    wrapped = lin.reshape(-1, 16).T          # [16, n/16]
    return np.tile(wrapped, (8, 1))          # [128, n/16]


def _prep_inputs(feat_miRNA, feat_gene, feat_drug, W_drug_disease, W_disease_drug,
                 W_drug, W_dis, mp_ins):
    """Marshal full inputs into per-core in_maps (no arithmetic on values)."""
    def pad_rows(a):
        a = np.ascontiguousarray(np.asarray(a, dtype=np.float32))
        if a.shape[0] >= R:
            return np.ascontiguousarray(a[:R])
        out = np.zeros((R, a.shape[1]), dtype=np.float32)
        out[: a.shape[0]] = a
        return out

    f_mi = pad_rows(feat_miRNA)
    f_ge = pad_rows(feat_gene)
    f_dr = pad_rows(feat_drug)
    wdd = np.ascontiguousarray(np.asarray(W_drug_disease, np.float32))
    wdg = np.ascontiguousarray(np.asarray(W_disease_drug, np.float32))
    wdrug = np.ascontiguousarray(np.asarray(W_drug, np.float32))
    wdis = np.ascontiguousarray(np.asarray(W_dis, np.float32))

    mp = np.asarray(mp_ins)
    assert mp.shape == (B_PAIRS, BAG, 4), mp.shape

    # X-path: within chunk c, gather slot j holds token (j%128)*CPBD + j//128
    jd = np.arange(CHD)
    tok_of_jd = (jd % P) * CPBD + (jd // P)

    in_maps = []
    for core in range(N_CORES):
        mp_core = mp[core * (B_PAIRS // N_CORES): (core + 1) * (B_PAIRS // N_CORES)]
        mp_core = mp_core.reshape(TOK, 4).astype(np.int16)

        idx_d = np.empty((P, 4, NCHD, CHD // 16), dtype=np.int16)
        for c in range(NCHD):
            t = c * CHD + tok_of_jd
            for k in range(4):
                idx_d[:, k, c, :] = _wrap_idx(mp_core[t, k])

        idx_p = np.empty((P, 4, NCHP, CHP // 16), dtype=np.int16)
        for c in range(NCHP):
            t = XT + c * CHP + np.arange(CHP)       # sequential tokens
            for k in range(4):
                idx_p[:, k, c, :] = _wrap_idx(mp_core[t, k])

        in_maps.append(
            {
                "feat_mi": f_mi,
                "feat_ge": f_ge,
                "feat_dr": f_dr,
                "w_dd": wdd,
                "w_dg": wdg,
                "w_drug": wdrug,
                "w_dis": wdis,
                "idx_d": idx_d,
                "idx_p": idx_p,
            }
        )
    return in_maps


def _numpy_fallback(feat_miRNA, feat_gene, feat_drug, W_drug_disease,
                    W_disease_drug, W_drug, W_dis, mp_ins):
    mi = np.asarray(feat_miRNA, np.float32)[mp_ins[:, :, 0]]
    g1 = np.asarray(feat_gene, np.float32)[mp_ins[:, :, 1]]
    g2 = np.asarray(feat_gene, np.float32)[mp_ins[:, :, 2]]
    dr = np.asarray(feat_drug, np.float32)[mp_ins[:, :, 3]]
    wdd = np.asarray(W_drug_disease, np.float32)
    wdg = np.asarray(W_disease_drug, np.float32)
    wdrug = np.asarray(W_drug, np.float32)
    wdis = np.asarray(W_dis, np.float32)
    dis = ((((mi + g1) * 0.5) @ wdd.T + g2) * 0.5 + dr) * 0.5
    drug = ((((dr + g2) * 0.5) @ wdg.T + g1) * 0.5 + mi) * 0.5
    return np.concatenate([drug @ wdrug.T, dis @ wdis.T], axis=2)


def kernel(**inputs):
    mp = np.asarray(inputs["mp_ins"])
    if mp.max() >= R or mp.min() < 0:
        # outside the spec's index range; fall back to exact host compute
        return _numpy_fallback(**inputs)

    from concourse.bass_utils import run_bass_kernel_spmd

    if "nc" not in _CACHE:
        _CACHE["nc"] = _build_module()
    nc = _CACHE["nc"]

    in_maps = _prep_inputs(**inputs)
    res = run_bass_kernel_spmd(nc, in_maps, core_ids=list(range(N_CORES)))
    outs = []
    for r in res.results:
        full = np.empty((TOK, H), dtype=np.float32)
        full[:XT] = np.asarray(r["out_d"]).astype(np.float32)  # exact upcast
        full[XT:] = r["out_p"]
        outs.append(full)
    return np.concatenate(outs, axis=0).reshape(B_PAIRS, BAG, H)


if __name__ == "__main__":
    import reference

    inputs = {k: np.asarray(v) for k, v in reference.setup_inputs().items()}
    expected = np.asarray(reference.reference(**inputs))
    actual = kernel(**inputs)
    err = np.abs(actual - expected).max() / (np.abs(expected).max() + 1e-9)
    print("max abs err (scaled):", err)
    rel = np.linalg.norm(actual - expected) / np.linalg.norm(expected)
    print("Relative error:", rel)
